# revision 14
# baseline (speedup 1.0000x reference)
"""Trainium2 Bass kernel for nn_MultiHeadDotProductAttention_19129784336436.

Leela-chess-style multi-head attention with "smolgen" attention-logit
generation. Data-parallel over batch: 1024 positions sharded as 128 per
NeuronCore across 8 cores; weights replicated.

Self-contained: hardcodes all shapes; host-side prep (transposes, padding,
bf16 casts, scale folding, shared_proj column permute) happens in numpy
inside kernel().
"""
import numpy as np
import ml_dtypes
from contextlib import ExitStack

import concourse.bass as bass
import concourse.tile as tile
from concourse import bacc, mybir
from concourse.bass_utils import run_bass_kernel_spmd

F32 = mybir.dt.float32
BF16 = mybir.dt.bfloat16
AF = mybir.ActivationFunctionType

N_CORES = 8
B, S, E = 1024, 79, 512
H, DH = 8, 64
SUM = 128
LL = S * S                 # 6241
BC = B // N_CORES          # 128 batch per core
ROWS = BC * S              # 10112
G = 8                      # batch group size in phase 2
N_GROUPS = BC // G         # 16
SUB = 8                    # batch sub size in phase 1 conv
N_SUBS = BC // SUB         # 16
TP = 80                    # t padded to DMA-transpose granule (16)

_CACHED = {}


def _bcast_free(ap, n):
    """Append a stride-0 free dim of size n to an AP (broadcast on read)."""
    return bass.AP(tensor=ap.tensor, offset=ap.offset, ap=list(ap.ap) + [[0, n]])


def _emit_smolgen(nc, tc, ctx, t, const):
    """Phase 1: smolgen logits for all BC batch elements -> DRAM scratch
    logits2[h, T, b, t] (bf16), T-major so phase 2 slab-loads [T, b-grp, t].
    """
    logits2 = t["logits2"]

    p1 = ctx.enter_context(tc.tile_pool(name="p1", bufs=2))
    p1c = ctx.enter_context(tc.tile_pool(name="p1c", bufs=1))

    c0w = const.tile([128, 4, 9, 16], BF16, tag="c0w")
    nc.sync.dma_start(out=c0w, in_=t["c0w"])
    c1w = const.tile([16, 9, 32], BF16, tag="c1w")
    nc.sync.dma_start(out=c1w, in_=t["c1w"])
    spw = const.tile([128, 4, 16], BF16, tag="spw")
    nc.sync.dma_start(out=spw, in_=t["spw"])
    g1w = const.tile([128, 56, 64], BF16, tag="g1w")
    nc.sync.dma_start(out=g1w, in_=t["g1w"])
    g2w = const.tile([64, 32], BF16, tag="g2w")
    nc.sync.dma_start(out=g2w, in_=t["g2w"])
    sumw = const.tile([80, 128], BF16, tag="sumw")
    nc.sync.dma_start(out=sumw, in_=t["sumw"])
    hww = const.tile([128, 8, 128], BF16, tag="hww")
    nc.sync.dma_start(out=hww, in_=t["hww"])
    sp2 = const.tile([128, LL], BF16, tag="sp2")
    nc.sync.dma_start(out=sp2, in_=t["sp2"])
    c0b = const.tile([16, 1], F32, tag="c0b")
    nc.sync.dma_start(out=c0b, in_=t["c0b"].rearrange("(a one) -> a one", one=1))
    c1b = const.tile([32, 1], F32, tag="c1b")
    nc.sync.dma_start(out=c1b, in_=t["c1b"].rearrange("(a one) -> a one", one=1))
    g1b = const.tile([64, 1], F32, tag="g1b")
    nc.sync.dma_start(out=g1b, in_=t["g1b"].rearrange("(a one) -> a one", one=1))
    cbias = const.tile([80, 1], F32, tag="cbias")
    nc.sync.dma_start(out=cbias, in_=t["cbias"].rearrange("(a one) -> a one", one=1))
    sumb = const.tile([128, 1], F32, tag="sumb")
    nc.sync.dma_start(out=sumb, in_=t["sumb"].rearrange("(a one) -> a one", one=1))
    hb = const.tile([128, 8], F32, tag="hb")
    nc.sync.dma_start(out=hb, in_=t["hb"])

    # combined features, transposed: rows 0-31 board_sum, 32-63 g, 64-79 spec
    combT = p1c.tile([80, BC], F32, tag="combT")

    # --- conv path ---
    with ExitStack() as cctx:
        ps_c0 = cctx.enter_context(
            tc.tile_pool(name="ps_c0", bufs=2, space="PSUM"))
        ps_c1 = cctx.enter_context(
            tc.tile_pool(name="ps_c1", bufs=2, space="PSUM"))
        canvases = [p1c.tile([16, SUB, 10, 10], BF16, tag=f"canv{i}",
                             name=f"canv{i}")
                    for i in range(2)]
        for cv in canvases:
            nc.vector.memset(cv, 0.0)
        xb = t["xboard"].rearrange("(kc p) b q -> p kc b q", p=128)
        for sub in range(N_SUBS):
            b0 = sub * SUB
            slab = p1.tile([128, 4, SUB, 10, 10], BF16, tag="slab")
            nc.sync.dma_start(out=slab, in_=xb[:, :, b0:b0 + SUB, :])
            c0ps = ps_c0.tile([16, SUB, 8, 8], F32, tag="c0ps")
            n = 0
            for kc in range(4):
                for off in range(9):
                    dy, dx = off // 3, off % 3
                    nc.tensor.matmul(
                        c0ps, c0w[:, kc, off, :],
                        slab[:, kc, :, dy:dy + 8, dx:dx + 8],
                        start=(n == 0), stop=(n == 35))
                    n += 1
            canv = canvases[sub % 2]
            nc.scalar.activation(canv[:, :, 1:9, 1:9], c0ps, AF.Relu,
                                 bias=c0b)
            c1ps = ps_c1.tile([32, SUB, 8, 8], F32, tag="c1ps")
            for off in range(9):
                dy, dx = off // 3, off % 3
                nc.tensor.matmul(c1ps, c1w[:, off, :],
                                 canv[:, :, dy:dy + 8, dx:dx + 8],
                                 start=(off == 0), stop=(off == 8))
            c1sb = p1.tile([32, SUB, 64], F32, tag="c1sb")
            nc.scalar.activation(c1sb, c1ps, AF.Relu, bias=c1b)
            nc.vector.reduce_sum(combT[0:32, b0:b0 + SUB], c1sb,
                                 axis=mybir.AxisListType.X)

    with ExitStack() as hctx:
        ps_sg = hctx.enter_context(
            tc.tile_pool(name="ps_sg", bufs=4, space="PSUM"))

        # --- g path ---
        gslab = p1.tile([128, 4, 14, BC], BF16, tag="gslab")
        nc.sync.dma_start(
            out=gslab, in_=t["xg"].rearrange("(kc p) s b -> p kc s b", p=128))
        g1ps = ps_sg.tile([64, BC], F32, tag="sg")
        for ck in range(56):
            s, kc = ck // 4, ck % 4
            nc.tensor.matmul(g1ps, g1w[:, ck, :], gslab[:, kc, s, :],
                             start=(ck == 0), stop=(ck == 55))
        g1sb = p1.tile([64, BC], BF16, tag="g1sb")
        nc.scalar.activation(g1sb, g1ps, AF.Relu, bias=g1b)
        g2ps = ps_sg.tile([64, BC], F32, tag="sg")
        nc.tensor.matmul(g2ps[32:64, :], g2w, g1sb, start=True, stop=True,
                         tile_position=(0, 32))
        nc.scalar.activation(combT[32:64, :], g2ps[32:64, :], AF.Relu,
                             bias=cbias[32:64, :])

        # --- special token ---
        spslab = p1.tile([128, 4, BC], BF16, tag="spslab")
        nc.sync.dma_start(
            out=spslab, in_=t["xspec"].rearrange("(kc p) b -> p kc b", p=128))
        spps = ps_sg.tile([80, BC], F32, tag="sg")
        for kc in range(4):
            nc.tensor.matmul(spps[64:80, :], spw[:, kc, :], spslab[:, kc, :],
                             start=(kc == 0), stop=(kc == 3),
                             tile_position=(0, 64))
        nc.scalar.activation(combT[64:80, :], spps[64:80, :], AF.Relu,
                             bias=cbias[64:80, :])

        # --- summary + heads + logits ---
        combTb = p1c.tile([80, BC], BF16, tag="combTb")
        nc.vector.tensor_copy(combTb, combT)
        psps = ps_sg.tile([128, BC], F32, tag="sg")
        nc.tensor.matmul(psps, sumw, combTb, start=True, stop=True)
        pssb = p1c.tile([128, BC], BF16, tag="pssb")
        nc.scalar.activation(pssb, psps, AF.Silu, bias=sumb)

        chunks = [(i * 474, 474) for i in range(13)] + [(13 * 474, 79)]
        hssbs = []
        for h in range(H):
            hsps = ps_sg.tile([128, BC], F32, tag="sg")
            nc.tensor.matmul(hsps, hww[:, h, :], pssb, start=True, stop=True)
            hssb = p1c.tile([128, BC], BF16, tag=f"hssb{h}", name=f"hssb{h}")
            nc.scalar.activation(hssb, hsps, AF.Silu, bias=hb[:, h:h + 1])
            hssbs.append(hssb)
        for c0_, cw in chunks:
            lgsb8 = p1.tile([BC, H, 474], BF16, tag="lgsb8")
            for h in range(H):
                lgps = ps_sg.tile([BC, 474], F32, tag="sg")
                nc.tensor.matmul(lgps[:, 0:cw], hssbs[h], sp2[:, c0_:c0_ + cw],
                                 start=True, stop=True)
                nc.scalar.activation(lgsb8[:, h, 0:cw], lgps[:, 0:cw], AF.Copy)
            # logits2[T, b, h, t]; src iterates (b, h, T, t)
            dst = logits2[c0_ // S:(c0_ + cw) // S, :, :, :].transpose(
                [1, 2, 0, 3])
            nc.sync.dma_start(
                out=dst,
                in_=lgsb8[:, :, 0:cw].rearrange("b h (T t) -> b h T t", t=S))


def _emit_main(nc, tc, ctx, t, const):
    """Phase 2: QKV, attention with smolgen bias, output projection.

    Q/K projections run on double groups (2*G=16 batch) to halve matmul
    count; attention runs per G=8 half. Softmax denominator rides the AV
    matmul as a 65th ones-column of V; normalization is fused into the
    PSUM evict. Attention output is transposed to [hd, (b,t)] slabs with
    one whole-tile DMA-transpose per batch element.
    """
    logits2, out = t["logits2"], t["out"]

    wq = const.tile([128, 4, 512], BF16, tag="wq")
    nc.sync.dma_start(out=wq, in_=t["wq"])
    wk = const.tile([128, 4, 512], BF16, tag="wk")
    nc.sync.dma_start(out=wk, in_=t["wk"])
    wv = const.tile([128, 4, 512], BF16, tag="wv")
    nc.sync.dma_start(out=wv, in_=t["wv"])
    wo = const.tile([128, 4, 512], BF16, tag="wo")
    nc.sync.dma_start(out=wo, in_=t["wo"])

    io = ctx.enter_context(tc.tile_pool(name="io", bufs=2))
    qk = ctx.enter_context(tc.tile_pool(name="qk", bufs=2))
    att = ctx.enter_context(tc.tile_pool(name="att", bufs=2))
    outp = ctx.enter_context(tc.tile_pool(name="outp", bufs=2))
    ps_mm = ctx.enter_context(tc.tile_pool(name="ps_mm", bufs=2, space="PSUM"))
    ps_sc = ctx.enter_context(tc.tile_pool(name="ps_sc", bufs=2, space="PSUM"))
    ps_av = ctx.enter_context(tc.tile_pool(name="ps_av", bufs=2, space="PSUM"))

    qbds = [qk.tile([128, 4, G, 2, S], BF16, tag=f"qbd{i}", name=f"qbd{i}",
                    bufs=1) for i in range(2)]
    for qb in qbds:
        nc.vector.memset(qb[0:64, :, :, 1, :], 0.0)
        nc.vector.memset(qb[64:128, :, :, 0, :], 0.0)

    NG = G * S               # 632 columns per group
    # n-chunks aligned to batch boundaries (6b + 2b) so the block-diag q
    # evicts stay affine
    nchunks = [(0, 474), (474, 158)]
    xqv = t["xqT"].rearrange("(kc p) r -> p kc r", p=128)
    xkvv = t["xkvT"].rearrange("(kc p) r -> p kc r", p=128)

    for g in range(N_GROUPS):
        col0 = g * NG
        xq_g = io.tile([128, 4, NG], BF16, tag="xq_g")
        nc.sync.dma_start(out=xq_g, in_=xqv[:, :, col0:col0 + NG])
        xkv_g = io.tile([128, 4, NG], BF16, tag="xkv_g")
        nc.sync.dma_start(out=xkv_g, in_=xkvv[:, :, col0:col0 + NG])

        # --- K projection (rows packed, M=128) ---
        kT_g = qk.tile([128, 4, NG], BF16, tag="kT_g")
        for m in range(4):
            for n0, nw in nchunks:
                mps = ps_mm.tile([128, 512], F32, tag="mm")
                for kc in range(4):
                    nc.tensor.matmul(
                        mps[:, 0:nw], wk[:, kc, m * 128:(m + 1) * 128],
                        xkv_g[:, kc, n0:n0 + nw],
                        start=(kc == 0), stop=(kc == 3))
                if m % 2 == 0:
                    nc.vector.tensor_copy(kT_g[:, m, n0:n0 + nw],
                                          mps[:, 0:nw])
                else:
                    nc.scalar.activation(kT_g[:, m, n0:n0 + nw],
                                         mps[:, 0:nw], AF.Copy)

        # --- Q projection into block-diagonal layout ---
        # qbd[p, m, b, half, t] = q[m*128+p, b*S+t] if p//64 == half else 0
        # static ping-pong pair: off-diagonal zeros memset once, persist
        qbd = qbds[g % 2]
        for m in range(4):
            for n0, nw in nchunks:
                b0, nb = n0 // S, nw // S
                mps = ps_mm.tile([128, 512], F32, tag="mm")
                for kc in range(4):
                    nc.tensor.matmul(
                        mps[:, 0:nw], wq[:, kc, m * 128:(m + 1) * 128],
                        xq_g[:, kc, n0:n0 + nw],
                        start=(kc == 0), stop=(kc == 3))
                nc.vector.tensor_copy(
                    qbd[0:64, m, b0:b0 + nb, 0, :],
                    mps[0:64, 0:nw].rearrange("p (b t) -> p b t", t=S))
                nc.scalar.activation(
                    qbd[64:128, m, b0:b0 + nb, 1, :],
                    mps[64:128, 0:nw].rearrange("p (b t) -> p b t", t=S),
                    AF.Copy)

        # --- V projection (per-b, [T, (h,d)] + ones column) ---
        vnat = qk.tile([S, G, 8, 65], BF16, tag="vnat")
        nc.vector.memset(vnat[:, :, :, 64:65], 1.0)
        for b in range(G):
            vps = ps_mm.tile([S, 512], F32, tag="mm")
            for kc in range(4):
                nc.tensor.matmul(
                    vps, xkv_g[:, kc, b * S:(b + 1) * S],
                    wv[:, kc, :], start=(kc == 0), stop=(kc == 3))
            nc.vector.tensor_copy(
                vnat[:, b, :, 0:64], vps.rearrange("T (h d) -> T h d", h=8))

        # --- smolgen logits slab [T, b, h, t] (one DMA) ---
        lgTall = att.tile([S, G, H, S], BF16, tag="lgTall")
        nc.sync.dma_start(out=lgTall,
                          in_=logits2[:, g * G:(g + 1) * G, :, :])

        # --- scoresT both heads per matmul (block-diag q) + add + exp ---
        expTs = []
        btriples = [(0, 3), (3, 3), (6, 2)]
        for hp in range(4):
            expT = att.tile([S, G, 2, S], BF16, tag="expT", bufs=4)
            expin = att.tile([S, G, 2, S], BF16, tag="expin")
            for b0, nb in btriples:
                psc = ps_sc.tile([S, 3, 2, S], F32, tag="sc")
                for bi in range(nb):
                    b = b0 + bi
                    nc.tensor.matmul(
                        psc[:, bi, :, :], kT_g[:, hp, b * S:(b + 1) * S],
                        qbd[:, hp, b, :, :], start=True, stop=True)
                nc.vector.tensor_add(
                    expin[:, b0:b0 + nb, :, :], psc[:, 0:nb, :, :],
                    lgTall[:, b0:b0 + nb, 2 * hp:2 * hp + 2, :])
            nc.scalar.activation(expT, expin, AF.Exp)
            expTs.append(expT)

        # --- AV (denominator = ones column); normalize on evict ---
        outT = outp.tile([128, 4, G, TP], BF16, tag="outT")
        for b in range(G):
            avpsE = ps_av.tile([S, 4, 65], F32, tag="avE")
            avpsO = ps_av.tile([S, 4, 65], F32, tag="avO")
            for h in range(H):
                eT = expTs[h // 2][:, b, h % 2, :]
                dst = avpsE if h % 2 == 0 else avpsO
                nc.tensor.matmul(dst[:, h // 2, :], eT, vnat[:, b, h, :],
                                 start=True, stop=True)
            recipE = att.tile([S, 4], F32, tag="recipE")
            recipO = att.tile([S, 4], F32, tag="recipO")
            nc.vector.reciprocal(recipE, avpsE[:, :, 64])
            nc.vector.reciprocal(recipO, avpsO[:, :, 64])
            osb = outp.tile([TP, 512], BF16, tag="osb", bufs=3)
            # col hd = h*64+d; h=2j -> j*128+d, h=2j+1 -> j*128+64+d
            ov = osb[0:S, :].rearrange("t (c h d) -> t c h d", c=4, h=2)
            nc.vector.tensor_mul(ov[:, :, 0, :], avpsE[:, :, 0:64],
                                 _bcast_free(recipE[:, :], 64))
            nc.vector.tensor_mul(ov[:, :, 1, :], avpsO[:, :, 0:64],
                                 _bcast_free(recipO[:, :], 64))
            nc.sync.dma_start_transpose(outT[:, :, b, :], osb)

        # --- output projection + store (skip pad rows t=79) ---
        outT_f = outT.rearrange("p c b t -> p (c b t)")
        rp = G * TP  # 640 padded rows per chunk c
        for mk in range(rp // 128):
            ops = ps_mm.tile([128, 512], F32, tag="mm")
            for c in range(4):
                nc.tensor.matmul(
                    ops,
                    outT_f[:, c * rp + mk * 128:c * rp + mk * 128 + 128],
                    wo[:, c, :], start=(c == 0), stop=(c == 3))
            fsb = outp.tile([128, 512], F32, tag="fsb", bufs=3)
            nc.scalar.activation(fsb, ops, AF.Copy)
            r0 = mk * 128
            for bb in range(r0 // TP, (r0 + 127) // TP + 1):
                lo = max(r0, bb * TP)
                hi = min(r0 + 128, bb * TP + S)
                if hi <= lo:
                    continue
                dr0 = (g * G + bb) * S + (lo - bb * TP)
                nc.scalar.dma_start(out=out[dr0:dr0 + hi - lo, :],
                                    in_=fsb[lo - r0:hi - r0, :])


def build_nc():
    nc = bacc.Bacc("TRN2", target_bir_lowering=False, debug=False)
    t = {}
    t["xqT"] = nc.dram_tensor("xqT", [E, ROWS], BF16,
                              kind="ExternalInput").ap()
    t["xkvT"] = nc.dram_tensor("xkvT", [E, ROWS], BF16,
                               kind="ExternalInput").ap()
    t["xboard"] = nc.dram_tensor("xboard", [E, BC * 100], BF16,
                                 kind="ExternalInput").ap()
    t["xspec"] = nc.dram_tensor("xspec", [E, BC], BF16,
                                kind="ExternalInput").ap()
    t["xg"] = nc.dram_tensor("xg", [E, 14 * BC], BF16,
                             kind="ExternalInput").ap()
    for nm, shp in [("wq", [128, 4, 512]), ("wk", [128, 4, 512]),
                    ("wv", [128, 4, 512]), ("wo", [128, 4, 512]),
                    ("spw", [128, 4, 16]), ("c0w", [128, 4, 9, 16]),
                    ("c1w", [16, 9, 32]), ("g1w", [128, 56, 64]),
                    ("g2w", [64, 32]), ("sumw", [80, 128]),
                    ("hww", [128, 8, 128]), ("sp2", [128, LL])]:
        t[nm] = nc.dram_tensor(nm, shp, BF16, kind="ExternalInput").ap()
    for nm, shp in [("c0b", [16]), ("c1b", [32]), ("g1b", [64]),
                    ("cbias", [80]), ("sumb", [128]), ("hb", [128, 8])]:
        t[nm] = nc.dram_tensor(nm, shp, F32, kind="ExternalInput").ap()
    t["out"] = nc.dram_tensor("out", [ROWS, E], F32,
                              kind="ExternalOutput").ap()

    with tile.TileContext(nc) as tc, ExitStack() as octx:
        const = octx.enter_context(tc.tile_pool(name="const", bufs=1))
        dram = octx.enter_context(
            tc.tile_pool(name="dram", bufs=1, space="DRAM"))
        t["xboard"] = t["xboard"].rearrange("e (b q) -> e b q", q=100)
        t["xg"] = t["xg"].rearrange("e (s b) -> e s b", b=BC)
        t["logits2"] = dram.tile([S, BC, H, S], BF16, tag="logits2",
                                 name="logits2")
        with ExitStack() as ctx1:
            _emit_smolgen(nc, tc, ctx1, t, const)
        with ExitStack() as ctx2:
            _emit_main(nc, tc, ctx2, t, const)
    nc.compile()
    return nc


def _prep(inputs):
    """Host-side tensor prep: weights shared across cores, x per-core."""
    bf = ml_dtypes.bfloat16
    f32 = np.float32
    w = {}

    def lhst(a):  # [512, N] -> [128, 4, N] with [p, kc, n] = a[kc*128+p, n]
        return np.ascontiguousarray(
            a.reshape(4, 128, -1).transpose(1, 0, 2)).astype(bf)

    w["wq"] = lhst(np.asarray(inputs["wq"], f32) / np.sqrt(DH))
    w["wk"] = lhst(np.asarray(inputs["wk"], f32))
    w["wv"] = lhst(np.asarray(inputs["wv"], f32))
    w["wo"] = lhst(np.asarray(inputs["wo"], f32))
    w["spw"] = lhst(np.asarray(inputs["sp_w"], f32))
    c0 = np.asarray(inputs["conv0_w"], f32).reshape(9, 4, 128, 16)
    w["c0w"] = np.ascontiguousarray(c0.transpose(2, 1, 0, 3)).astype(bf)
    c1 = np.asarray(inputs["conv1_w"], f32).reshape(9, 16, 32)
    w["c1w"] = np.ascontiguousarray(c1.transpose(1, 0, 2)).astype(bf)
    g1 = np.asarray(inputs["g1_w"], f32).reshape(56, 128, 64)
    w["g1w"] = np.ascontiguousarray(g1.transpose(1, 0, 2)).astype(bf)
    w["g2w"] = np.asarray(inputs["g2_w"], f32).astype(bf)
    sw = np.asarray(inputs["sum_w"], f32)
    w["sumw"] = np.concatenate(
        [sw[16:48] / 64.0, sw[48:80], sw[0:16]], axis=0).astype(bf)
    w["hww"] = np.ascontiguousarray(
        np.asarray(inputs["head_w"], f32).transpose(1, 0, 2)).astype(bf)
    sp = np.asarray(inputs["shared_proj"], f32).reshape(SUM, S, S)
    w["sp2"] = np.ascontiguousarray(
        sp.transpose(0, 2, 1).reshape(SUM, LL)).astype(bf)
    w["c0b"] = np.ascontiguousarray(np.asarray(inputs["conv0_b"], f32))
    w["c1b"] = np.ascontiguousarray(np.asarray(inputs["conv1_b"], f32))
    w["g1b"] = np.ascontiguousarray(np.asarray(inputs["g1_b"], f32))
    w["cbias"] = np.concatenate([np.zeros(32, f32),
                                 np.asarray(inputs["g2_b"], f32),
                                 np.asarray(inputs["sp_b"], f32)])
    w["sumb"] = np.ascontiguousarray(np.asarray(inputs["sum_b"], f32))
    w["hb"] = np.ascontiguousarray(np.asarray(inputs["head_b"], f32).T)

    xq = np.asarray(inputs["inputs_q"], f32)
    xkv = np.asarray(inputs["inputs_kv"], f32)
    maps = []
    for c in range(N_CORES):
        xq_c = xq[c * BC:(c + 1) * BC]    # [BC, S, E]
        xkv_c = xkv[c * BC:(c + 1) * BC]
        m = dict(w)
        m["xqT"] = np.ascontiguousarray(
            xq_c.transpose(2, 0, 1).reshape(E, ROWS)).astype(bf)
        m["xkvT"] = np.ascontiguousarray(
            xkv_c.transpose(2, 0, 1).reshape(E, ROWS)).astype(bf)
        board = np.zeros((E, BC, 10, 10), f32)
        board[:, :, 1:9, 1:9] = xq_c[:, 1:65, :].transpose(2, 0, 1).reshape(
            E, BC, 8, 8)
        m["xboard"] = board.reshape(E, BC * 100).astype(bf)
        m["xspec"] = np.ascontiguousarray(xq_c[:, 0, :].T).astype(bf)
        m["xg"] = np.ascontiguousarray(
            xq_c[:, 65:79, :].transpose(2, 1, 0).reshape(E, 14 * BC)).astype(bf)
        maps.append(m)
    return maps


def kernel(**inputs) -> np.ndarray:
    if "nc" not in _CACHED:
        _CACHED["nc"] = build_nc()
    nc = _CACHED["nc"]
    in_maps = _prep(inputs)
    res = run_bass_kernel_spmd(nc, in_maps, list(range(N_CORES)))
    outs = [res.results[c]["out"].reshape(BC, S, E) for c in range(N_CORES)]
    return np.concatenate(outs, axis=0)
